# revision 37
# baseline (speedup 1.0000x reference)
"""GCN + DiffPool kernel for Trainium2, data-parallel over graphs across 8 NeuronCores.

Model (per graph, n=150 nodes):
  Z1 = relu(An @ (x @ W1) + b1)          An = D^-1/2 (A+I) D^-1/2
  Z2 = relu(An @ (Z1 @ W2) + b2)
  S  = softmax(An @ (Z2 @ Wa) + ba)      [n, 25]
  Zp = S^T @ Z2 ; Ap = S^T @ (A @ S)
  H  = relu(Anp @ (Zp @ Wp) + bp)        pooled GCN, 25 cluster-nodes
  logits = (sum_rows H) @ Wc + bc

Sharding: 64 graphs -> 8 devices x 8 graphs; block-diagonal adjacency means each
device only gets its 8 graphs' 150x150 blocks and its node rows of x.

Structure (all graph normalization host-precomputed):
  - Host ships an = (A+I) .* d_col (column-normalized, bf16) AND raw ah = A+I,
    plus x pre-scaled by d rows, d node-major (f32, for m2/v drains) and
    deg-1 node-major (bf16). No degree reduce / rsqrt / dT broadcast on device.
  - Deferred normalization: An @ M = d .* (an^T contraction) with the row
    factor folded into the moving operand m = d.*(Z@W) at drain time.
  - v and Y = Z2@Wp share their stationary (z2 slices): one matmul with
    rhs = [Wa | Wp] (89 cols).
  - Fused pooling matmul: out[25, g, 90] = S^T @ [Y | AS | deg-1] gives
    Zp@Wp (cluster-major), Ap, and pooled degrees (rowsum(Ap) = S^T(deg-1))
    in one 16-matmul set; dp = rsqrt(1 + col89) via quake+Newton on DVE.
  - H = relu(dpT .* ((Ap+I) @ (dp .* ZpWp)) + bp) with dpT partition-broadcast
    via one PE transpose + selector matmuls; logits single bf16 matmul.

On-device layout convention:
  fm (feature-major): [feat_part, graph, node]  - used for W-multiplies (lhsT)
  nm (node-major):    [node_part, graph, feat]  - used for A-multiplies
Node dim (150) splits into partition chunks c0=[0:128], c1=[128:150].
"""

import numpy as np

import concourse.bass as bass
import concourse.mybir as mybir
import concourse.tile as tile
from concourse import bacc
from concourse.bass_utils import run_bass_kernel_spmd

F32 = mybir.dt.float32
BF16 = mybir.dt.bfloat16
AF = mybir.ActivationFunctionType
AL = mybir.AluOpType
U32 = mybir.dt.uint32

MMDT = BF16

N_NODES = 9600
N_FEAT = 128
HIDDEN = 64
CLUSTERS = 25
NUM_CLASSES = 10
B_GRAPHS = 64
NPG = 150            # nodes per graph
DEV = 8              # devices
GPD = 8              # graphs per device
C0, C1 = 128, 22     # node partition chunks (128 + 22 = 150)
VYC = CLUSTERS + HIDDEN          # 89: [Wa | Wp] fused free dim
CATC = HIDDEN + CLUSTERS + 1     # 90: [Y | AS | degm1] fused free dim

_CACHE = {}

# wpk (bf16) packed-constant column offsets
WP_W1 = 0                       # [128, 64]
WP_W2 = WP_W1 + HIDDEN          # [64, 64]
WP_WAP = WP_W2 + HIDDEN         # [64, 89] = [Wa | Wp] fused
WP_WC = WP_WAP + VYC            # [64, 10]
WP_DM0 = WP_WC + NUM_CLASSES    # [128, 8] deg-1 chunk0, node-major
WP_DM1 = WP_DM0 + GPD           # [22, 8] deg-1 chunk1
WP_ID25 = WP_DM1 + GPD          # [25, 25] bf16 identity (dp transpose)
WP_COLS = WP_ID25 + CLUSTERS

# fpk (f32) packed-constant column offsets
FP_BC = 0                       # [8, 10] bc broadcast over graphs
FP_B1 = FP_BC + NUM_CLASSES     # [64, 1]
FP_B2 = FP_B1 + 1
FP_BP = FP_B2 + 1
FP_BA = FP_BP + 1               # [128, 25] ba broadcast over partitions
FP_ID25 = FP_BA + CLUSTERS      # [25, 25] identity (ahp build)
FP_D0 = FP_ID25 + CLUSTERS      # [128, 8] d chunk0, node-major
FP_D1 = FP_D0 + GPD             # [22, 8] d chunk1
FP_DI0 = FP_D1 + GPD            # [128, 8] 1/d = sqrt(deg) chunk0
FP_DI1 = FP_DI0 + GPD           # [22, 8] chunk1
FP_COLS = FP_DI1 + GPD


def _chunk(c):
    return (0, C0) if c == 0 else (C0, C1)


def build_nc():
    nc = bacc.Bacc("TRN2", target_bir_lowering=False, debug=False, num_devices=DEV)

    def din(name, shape, dt=F32):
        return nc.dram_tensor(name, shape, dt, kind="ExternalInput").ap()

    xT = din("xT", [N_FEAT, GPD, NPG], MMDT)     # d .* x, feature-major
    an0 = din("an0", [C0, GPD, NPG], MMDT)       # (A+I).*d_col rows 0:128
    an1 = din("an1", [C1 + 1, GPD, NPG], MMDT)   # rows 128:150 + ones row 22
    wpk = din("wpk", [N_FEAT, WP_COLS], MMDT)
    fpk = din("fpk", [N_FEAT, FP_COLS], F32)
    selp = din("selp", [GPD, GPD * HIDDEN], MMDT)  # one-hot row selectors
    out = nc.dram_tensor("out", [GPD, NUM_CLASSES], F32, kind="ExternalOutput").ap()

    with tile.TileContext(nc) as tc:
        with (
            tc.tile_pool(name="cst", bufs=1) as cst,
            tc.tile_pool(name="act", bufs=1) as act,
            tc.tile_pool(name="ps", bufs=7, space="PSUM") as ps,
            tc.tile_pool(name="pst", bufs=1, space="PSUM") as pst,
        ):
            # ---- input DMAs. xT + wpk head the m1 chain; an gates z1;
            # ah only gates AS (mid-kernel); selp only the dpT broadcast. ----
            s_xT = cst.tile([N_FEAT, GPD, NPG], MMDT, tag="xT")
            nc.sync.dma_start(out=s_xT[:], in_=xT)
            s_wpk = cst.tile([N_FEAT, WP_COLS], MMDT, tag="wpk")
            nc.scalar.dma_start(out=s_wpk[:], in_=wpk)
            s_an0 = cst.tile([C0, GPD, NPG], MMDT, tag="an0")
            nc.sync.dma_start(out=s_an0[:], in_=an0)
            s_an1 = cst.tile([C1 + 1, GPD, NPG], MMDT, tag="an1")
            nc.scalar.dma_start(out=s_an1[:], in_=an1)
            s_fpk = cst.tile([N_FEAT, FP_COLS], F32, tag="fpk")
            nc.scalar.dma_start(out=s_fpk[:], in_=fpk)
            s_selp = cst.tile([GPD, GPD * HIDDEN], MMDT, tag="selp")
            nc.scalar.dma_start(out=s_selp[:], in_=selp)

            s_W1 = s_wpk[:, WP_W1:WP_W1 + HIDDEN]
            s_W2 = s_wpk[0:HIDDEN, WP_W2:WP_W2 + HIDDEN]
            s_WaP = s_wpk[0:HIDDEN, WP_WAP:WP_WAP + VYC]
            s_Wc = s_wpk[0:HIDDEN, WP_WC:WP_WC + NUM_CLASSES]
            s_dm = [s_wpk[:, WP_DM0:WP_DM0 + GPD],
                    s_wpk[0:C1, WP_DM1:WP_DM1 + GPD]]
            s_id25b = s_wpk[0:CLUSTERS, WP_ID25:WP_ID25 + CLUSTERS]
            s_bc = s_fpk[0:GPD, FP_BC:FP_BC + NUM_CLASSES]
            s_b1 = s_fpk[0:HIDDEN, FP_B1:FP_B1 + 1]
            s_b2 = s_fpk[0:HIDDEN, FP_B2:FP_B2 + 1]
            s_bp = s_fpk[0:HIDDEN, FP_BP:FP_BP + 1]
            s_baB = s_fpk[:, FP_BA:FP_BA + CLUSTERS]
            s_id25 = s_fpk[0:CLUSTERS, FP_ID25:FP_ID25 + CLUSTERS]
            s_d = [s_fpk[:, FP_D0:FP_D0 + GPD],
                   s_fpk[0:C1, FP_D1:FP_D1 + GPD]]
            s_di = [s_fpk[:, FP_DI0:FP_DI0 + GPD],
                    s_fpk[0:C1, FP_DI1:FP_DI1 + GPD]]

            # quake rsqrt constants (dp only)
            qk1 = act.tile([CLUSTERS, 1], U32, tag="qk1")
            nc.vector.memset(qk1[:], 1)
            qkm = act.tile([CLUSTERS, 1], U32, tag="qkm")
            nc.vector.memset(qkm[:], 0x5F3759DF)

            # rhscat: [Y | AS | degm1] node-major, rhs of the fused pooling
            # matmul. degm1 lands first (host constant, off critical path).
            rhscat = []
            for c, cn in ((0, C0), (1, C1)):
                t = act.tile([cn, GPD, CATC], MMDT, tag=f"rhscat{c}")
                nc.vector.tensor_copy(
                    t[:, :, HIDDEN + CLUSTERS:CATC],
                    s_dm[c][:][:, :, None])
                rhscat.append(t)

            # ---- helpers ---------------------------------------------------
            def w_mult_nm(lhs_fm, w, kdim, fout, name, dscale=True):
                """m = [d .*] (Z @ W), node-major chunks. lhsT = fm slice."""
                outs = []
                for c, cn in ((0, C0), (1, C1)):
                    off, _ = _chunk(c)
                    p = ps.tile([cn, GPD, fout], F32, tag="ps")
                    for g in range(GPD):
                        nc.tensor.matmul(p[:, g, :],
                                         lhs_fm[0:kdim, g, off:off + cn],
                                         w, start=True, stop=True)
                    o = act.tile([cn, GPD, fout], MMDT, tag=f"{name}{c}")
                    if dscale:
                        dbc = s_d[c][:][:, :, None].broadcast_to((cn, GPD, fout))
                        nc.vector.tensor_mul(o[:], p[:], dbc)
                    else:
                        nc.scalar.copy(o[:], p[:])
                    outs.append(o)
                return outs

            def an_mult_fm(m_nm, bias, name):
                """fm out [64, g, 150] = relu(An @ m + bias), graph-paired
                PSUM tiles, ACT-only drain."""
                o = act.tile([HIDDEN, GPD, NPG], MMDT, tag=name)
                for q in range(GPD // 2):
                    p = ps.tile([HIDDEN, 2, NPG], F32, tag="ps")
                    for k in range(2):
                        g = 2 * q + k
                        nc.tensor.matmul(p[:, k, :], m_nm[0][:, g, :],
                                         s_an0[:, g, :], start=True, stop=False)
                        nc.tensor.matmul(p[:, k, :], m_nm[1][0:C1, g, :],
                                         s_an1[0:C1, g, :], start=False, stop=True)
                    nc.scalar.activation(o[:, 2 * q:2 * q + 2, :], p[:],
                                         AF.Relu, bias=bias)
                return o

            # ---- encoder ---------------------------------------------------
            # m1 = (d.*x) @ W1: x pre-scaled on host, plain ACT drain
            m1 = w_mult_nm(s_xT, s_W1, N_FEAT, HIDDEN, "m1", dscale=False)
            z1 = an_mult_fm(m1, s_b1, "z1")
            m2 = w_mult_nm(z1, s_W2, HIDDEN, HIDDEN, "m2")
            z2 = an_mult_fm(m2, s_b2, "z2")

            # ---- fused v | Y: rhs = [Wa | Wp], shared stationary z2 --------
            # v = d.*(z2@Wa) (+ba row), Y = z2@Wp (raw, into rhscat cols 0:64)
            # psum split by graph half so each tile stays within one 2KiB bank
            HG = GPD // 2
            v = []
            for c, cn in ((0, C0), (1, C1)):
                off, _ = _chunk(c)
                rows = cn + (1 if c == 1 else 0)
                o = act.tile([rows, GPD, CLUSTERS], MMDT, tag=f"v{c}")
                if c == 1:
                    bsrc = s_baB[0:rows, 0:CLUSTERS][:, None, :] \
                        .broadcast_to((rows, GPD, CLUSTERS))
                    nc.vector.tensor_copy(o[:], bsrc)
                for h in range(2):
                    gl, gh = h * HG, (h + 1) * HG
                    p = ps.tile([cn, HG, 128], F32, tag="ps")
                    for k in range(HG):
                        nc.tensor.matmul(p[:, k, 0:VYC],
                                         z2[0:HIDDEN, gl + k, off:off + cn],
                                         s_WaP, start=True, stop=True)
                    dbc = s_d[c][:, gl:gh][:, :, None] \
                        .broadcast_to((cn, HG, CLUSTERS))
                    nc.vector.tensor_mul(o[0:cn, gl:gh, :],
                                         p[:, :, 0:CLUSTERS], dbc)
                    nc.scalar.copy(rhscat[c][:, gl:gh, 0:HIDDEN],
                                   p[:, :, CLUSTERS:VYC])
                v.append(o)

            # ---- assignment: S = softmax(An @ v + ba), nm.  ba rides the
            # an1 ones-row x v1 ba-row rank-1 term inside the matmul. -------
            s_P, s_S = [], []
            for mc, mn in ((0, C0), (1, C1)):
                moff, _ = _chunk(mc)
                p = ps.tile([mn, GPD, CLUSTERS], F32, tag="ps")
                for g in range(GPD):
                    nc.tensor.matmul(p[:, g, :], s_an0[:, g, moff:moff + mn],
                                     v[0][:, g, :], start=True, stop=False)
                    nc.tensor.matmul(p[:, g, :], s_an1[0:C1 + 1, g, moff:moff + mn],
                                     v[1][0:C1 + 1, g, :], start=False, stop=True)
                s_P.append(p)
                st = act.tile([mn, GPD, CLUSTERS], MMDT, tag=f"s{mc}")
                s_S.append(st)
            # softmax drains by graph half so AS for the first graphs starts
            # one half-chain earlier
            for h in range(2):
                gl, gh = h * HG, (h + 1) * HG
                for mc, mn in ((0, C0), (1, C1)):
                    p, s = s_P[mc], s_S[mc]
                    e = act.tile([mn, HG, CLUSTERS], F32, tag=f"e{mc}{h}")
                    nc.scalar.activation(e[:], p[:, gl:gh, :], AF.Exp)
                    ssum = act.tile([mn, HG], F32, tag=f"ssum{mc}{h}")
                    nc.vector.reduce_sum(out=ssum[:], in_=e[:],
                                         axis=mybir.AxisListType.X)
                    rs = act.tile([mn, HG], F32, tag=f"rs{mc}{h}")
                    nc.vector.reciprocal(rs[:], ssum[:])
                    nc.vector.tensor_mul(
                        s[:, gl:gh, :], e[:],
                        rs[:][:, :, None].broadcast_to((mn, HG, CLUSTERS)))

            # ---- AS = A @ S = dinv .* (an-contraction) - S ----------------
            for mc, mn in ((0, C0), (1, C1)):
                moff, _ = _chunk(mc)
                p = ps.tile([mn, GPD, CLUSTERS], F32, tag="ps")
                for g in range(GPD):
                    nc.tensor.matmul(p[:, g, :], s_an0[:, g, moff:moff + mn],
                                     s_S[0][:, g, :], start=True, stop=False)
                    nc.tensor.matmul(p[:, g, :], s_an1[0:C1, g, moff:moff + mn],
                                     s_S[1][:, g, :], start=False, stop=True)
                for h in range(2):
                    gl, gh = h * HG, (h + 1) * HG
                    dib = s_di[mc][:, gl:gh][:, :, None] \
                        .broadcast_to((mn, HG, CLUSTERS))
                    nc.vector.tensor_mul(
                        rhscat[mc][:, gl:gh, HIDDEN:HIDDEN + CLUSTERS],
                        p[:, gl:gh, :], dib)
                    nc.vector.tensor_tensor(
                        rhscat[mc][:, gl:gh, HIDDEN:HIDDEN + CLUSTERS],
                        rhscat[mc][:, gl:gh, HIDDEN:HIDDEN + CLUSTERS],
                        s_S[mc][:, gl:gh, :], AL.subtract)

            # ---- fused pooling matmul: [ZpWp | Ap | degp] = S^T @ rhscat --
            # psum padded to 128 cols (so no graph's 90-col slice crosses a
            # 2KiB bank boundary) and split by graph half (one bank each).
            p_cat = []
            for h in range(2):
                gl = h * HG
                p = ps.tile([CLUSTERS, HG, 128], F32, tag="ps")
                for k in range(HG):
                    g = gl + k
                    nc.tensor.matmul(p[:, k, 0:CATC], s_S[0][:, g, :],
                                     rhscat[0][:, g, :], start=True, stop=False)
                    nc.tensor.matmul(p[:, k, 0:CATC], s_S[1][:, g, :],
                                     rhscat[1][:, g, :], start=False, stop=True)
                p_cat.append(p)

            # ---- per-half dp quake chains (DVE), emitted right after the
            # cat matmuls so h0's chain overlaps h1's matmuls --------------
            dps = []
            for h in range(2):
                degp = act.tile([CLUSTERS, HG], F32, tag=f"degp{h}")
                nc.vector.tensor_scalar_add(
                    degp[:], p_cat[h][:, :, HIDDEN + CLUSTERS], 1.0)
                dpw = act.tile([CLUSTERS, HG], F32, tag=f"dpw{h}")
                nc.vector.tensor_tensor(dpw[:].bitcast(U32),
                                        degp[:].bitcast(U32),
                                        qk1[:].broadcast_to((CLUSTERS, HG)),
                                        AL.logical_shift_right)
                nc.vector.tensor_tensor(dpw[:].bitcast(U32),
                                        qkm[:].broadcast_to((CLUSTERS, HG)),
                                        dpw[:].bitcast(U32), AL.subtract)
                dpw2 = act.tile([CLUSTERS, HG], F32, tag=f"dpw2{h}")
                nc.vector.tensor_mul(dpw2[:], dpw[:], dpw[:])
                nc.vector.tensor_mul(dpw2[:], dpw2[:], degp[:])
                nc.vector.tensor_scalar(dpw2[:], dpw2[:], -0.5, 1.5,
                                        AL.mult, AL.add)
                dp = act.tile([CLUSTERS, HG], MMDT, tag=f"dp{h}")
                nc.vector.tensor_mul(dp[:], dpw[:], dpw2[:])
                dps.append(dp)

            # ---- per-half tails: dpT broadcast, ahp/mp, H, relu, readout.
            # h1's quake chain runs on DVE while h0's tail occupies the PE. -
            s_G = act.tile([HIDDEN, GPD], MMDT, tag="g")
            id25b = s_id25[:, None, :].broadcast_to((CLUSTERS, HG, CLUSTERS))
            for h in range(2):
                gl, gh = h * HG, (h + 1) * HG
                dp = dps[h]
                p_dpt = pst.tile([HG, CLUSTERS], MMDT, tag="p2")
                nc.tensor.transpose(p_dpt[:], dp[:], s_id25b)
                dpTrow = act.tile([HG, CLUSTERS], MMDT, tag=f"dpTrow{h}")
                nc.vector.tensor_copy(dpTrow[:], p_dpt[:])
                p_dpb = ps.tile([HIDDEN, HG, CLUSTERS], F32, tag="ps")
                for k in range(HG):
                    nc.tensor.matmul(p_dpb[:, k, :],
                                     s_selp[0:HG, k * HIDDEN:(k + 1) * HIDDEN],
                                     dpTrow[:], start=True, stop=True)
                s_dpT = act.tile([HIDDEN, HG, CLUSTERS], F32, tag=f"dpt{h}")
                nc.scalar.copy(s_dpT[:], p_dpb[:])
                ahp = act.tile([CLUSTERS, HG, CLUSTERS], MMDT, tag=f"ahp{h}")
                nc.vector.tensor_add(ahp[:],
                                     p_cat[h][:, :, HIDDEN:HIDDEN + CLUSTERS],
                                     id25b)
                mp = act.tile([CLUSTERS, HG, HIDDEN], MMDT, tag=f"mp{h}")
                nc.vector.tensor_mul(
                    mp[:], p_cat[h][:, :, 0:HIDDEN],
                    dp[:][:, :, None].broadcast_to((CLUSTERS, HG, HIDDEN)))
                p_h = ps.tile([HIDDEN, HG, CLUSTERS], F32, tag="ps")
                for k in range(HG):
                    nc.tensor.matmul(p_h[:, k, :], mp[:, k, :], ahp[:, k, :],
                                     start=True, stop=True)
                th = act.tile([HIDDEN, HG, CLUSTERS], F32, tag=f"th{h}")
                nc.vector.tensor_mul(th[:], p_h[:], s_dpT[:])
                s_H = act.tile([HIDDEN, HG, CLUSTERS], F32, tag=f"h{h}")
                nc.scalar.activation(s_H[:], th[:], AF.Relu, bias=s_bp)
                with nc.allow_low_precision(reason="bf16 G readout for 1-mm logits"):
                    nc.vector.reduce_sum(out=s_G[:, gl:gh], in_=s_H[:],
                                         axis=mybir.AxisListType.X)

            p_l = ps.tile([GPD, NUM_CLASSES], F32, tag="ps")
            nc.tensor.matmul(p_l[:], s_G[:], s_Wc, start=True, stop=True)
            s_out = act.tile([GPD, NUM_CLASSES], F32, tag="logits")
            nc.vector.tensor_add(s_out[:], p_l[:], s_bc)
            nc.sync.dma_start(out=out, in_=s_out[:])

    nc.compile()
    return nc


def make_in_maps(x, a, W1, b1, W2, b2, Wa, ba, Wp, bp, Wc, bc):
    import ml_dtypes
    npmm = np.dtype(ml_dtypes.bfloat16) if MMDT == BF16 else np.dtype(np.float32)

    x = np.ascontiguousarray(np.asarray(x, dtype=np.float32))
    a = np.asarray(a, dtype=np.float32)

    # diagonal 150x150 blocks of the batch adjacency, self-loops pre-added
    ab = a.reshape(B_GRAPHS, NPG, B_GRAPHS, NPG)
    blocks = ab[np.arange(B_GRAPHS), :, np.arange(B_GRAPHS), :]  # [64, 150, 150]
    blocks = blocks + np.eye(NPG, dtype=np.float32)[None]

    deg = blocks.sum(axis=2)                       # [64, 150]
    d = 1.0 / np.sqrt(np.maximum(deg, 1e-12))      # [64, 150]
    dinv = np.sqrt(np.maximum(deg, 1e-12))         # [64, 150]
    degm1 = (deg - 1.0)                            # [64, 150]

    an_blocks = (blocks * d[:, None, :]).astype(npmm)   # column-normalized

    # x rows pre-scaled by d
    dflat = d.reshape(-1)                          # [9600]
    xs = x * dflat[:, None]

    wpk = np.zeros((N_FEAT, WP_COLS), npmm)
    wpk[:, WP_W1:WP_W1 + HIDDEN] = np.asarray(W1, np.float32).astype(npmm)
    wpk[0:HIDDEN, WP_W2:WP_W2 + HIDDEN] = np.asarray(W2, np.float32).astype(npmm)
    wpk[0:HIDDEN, WP_WAP:WP_WAP + CLUSTERS] = np.asarray(Wa, np.float32).astype(npmm)
    wpk[0:HIDDEN, WP_WAP + CLUSTERS:WP_WAP + VYC] = \
        np.asarray(Wp, np.float32).astype(npmm)
    wpk[0:HIDDEN, WP_WC:WP_WC + NUM_CLASSES] = np.asarray(Wc, np.float32).astype(npmm)
    wpk[0:CLUSTERS, WP_ID25:WP_ID25 + CLUSTERS] = np.eye(CLUSTERS, dtype=npmm)

    fpk = np.zeros((N_FEAT, FP_COLS), np.float32)
    fpk[0:GPD, FP_BC:FP_BC + NUM_CLASSES] = np.asarray(bc, np.float32)[None, :]
    fpk[0:HIDDEN, FP_B1] = np.asarray(b1, np.float32)
    fpk[0:HIDDEN, FP_B2] = np.asarray(b2, np.float32)
    fpk[0:HIDDEN, FP_BP] = np.asarray(bp, np.float32)
    fpk[:, FP_BA:FP_BA + CLUSTERS] = np.asarray(ba, np.float32)[None, :]
    fpk[0:CLUSTERS, FP_ID25:FP_ID25 + CLUSTERS] = np.eye(CLUSTERS, dtype=np.float32)

    selp = np.zeros((GPD, GPD * HIDDEN), npmm)
    for g in range(GPD):
        selp[g, g * HIDDEN:(g + 1) * HIDDEN] = 1.0

    in_maps = []
    for dev in range(DEV):
        gs = slice(dev * GPD, (dev + 1) * GPD)
        xd = xs[dev * GPD * NPG:(dev + 1) * GPD * NPG]      # [1200, 128]
        xTd = np.ascontiguousarray(xd.T).reshape(N_FEAT, GPD, NPG).astype(npmm)

        anb = np.ascontiguousarray(an_blocks[gs].transpose(1, 0, 2))  # [150,8,150]
        an1t = np.ones((C1 + 1, GPD, NPG), npmm)
        an1t[0:C1] = anb[C0:]

        dd = d[gs]                                           # [8, 150]
        di = dinv[gs]                                        # [8, 150]
        dm = degm1[gs]                                       # [8, 150]
        fpkd = fpk.copy()
        fpkd[0:C0, FP_D0:FP_D0 + GPD] = dd[:, 0:C0].T
        fpkd[0:C1, FP_D1:FP_D1 + GPD] = dd[:, C0:].T
        fpkd[0:C0, FP_DI0:FP_DI0 + GPD] = di[:, 0:C0].T
        fpkd[0:C1, FP_DI1:FP_DI1 + GPD] = di[:, C0:].T
        wpkd = wpk.copy()
        wpkd[0:C0, WP_DM0:WP_DM0 + GPD] = dm[:, 0:C0].T.astype(npmm)
        wpkd[0:C1, WP_DM1:WP_DM1 + GPD] = dm[:, C0:].T.astype(npmm)

        in_maps.append(dict(
            xT=xTd,
            an0=np.ascontiguousarray(anb[:C0]),
            an1=an1t,
            wpk=wpkd,
            fpk=fpkd,
            selp=selp,
        ))
    return in_maps


def kernel(x, a, seg_ids, num_graphs, W1, b1, W2, b2, Wa, ba, Wp, bp, Wc, bc,
           trace=False):
    if "nc" not in _CACHE:
        _CACHE["nc"] = build_nc()
    nc = _CACHE["nc"]
    in_maps = make_in_maps(x, a, W1, b1, W2, b2, Wa, ba, Wp, bp, Wc, bc)
    res = run_bass_kernel_spmd(nc, in_maps, core_ids=list(range(DEV)), trace=trace)
    logits = np.concatenate([r["out"] for r in res.results], axis=0)
    if trace:
        return logits, res
    return logits
